# revision 35
# baseline (speedup 1.0000x reference)
"""Multi-head attention (B=4, S=2048, D=1024, H=16) on 8 Trainium2 cores.

Sharding: data parallel on batch (4) x tensor parallel on heads (2 halves of
8 heads). Core c handles batch c//2 and head-half c%2: column-parallel
w_q/w_k/w_v (512 out dims), local attention over its 8 heads, row-parallel
w_o (its 512 hd columns) producing a full [2048, 1024] partial that the host
sums across the two halves (plus b_o and the folded b_v @ w_o.T term).

On-device layout is feature-on-partitions throughout ("transposed"):
  qP/kP: per-(dt, block) [128, 512] bf16 tiles (projection form B)
  scores S.T: [keys, queries] via paired K=64 matmuls (head pair at PE row
  offsets 0/64 with tile_position) into a 2-bank PSUM tile, one wide exp ACT
  AV: O.T accumulation with V_aug ones-column producing row sums; normalize
  via DVE fast reciprocal + GpSimd partition-broadcast.

Bias handling (no bias matmuls at all):
  b_k drops: q.b_k is constant per query -> softmax-invariant.
  b_q folds into the Q writeback as a DVE per-partition tensor_scalar add.
  b_v folds into the host-side add: attn rows sum to 1, so its effect is
  the constant vector b_v @ w_o.T.

All persistent tensors are split into per-(dt, block) [128, 512] tiles so
the Tile dep tracker never sees false write-read sharing between disjoint
slices (a single big tile serializes later readers on the freshest write).

Software pipelined schedule: steady state is one exp ACT per kt step (ACT
~100% busy); each combo's first score pair + exp are emitted BEFORE the
previous combo's final-AV/normalize tail so ACT never idles at boundaries.
Input DMAs are column-chunked: the Sync queue carries the T0-critical
wk/kT/wq/qT-qb0 stream, the GpSimd queue carries wv/vT gated behind a probe
copy of the last qb0 chunk so it cannot steal bandwidth from the ramp.
"""

import time
from collections import deque
from contextlib import ExitStack

import ml_dtypes
import numpy as np

import concourse.bass as bass
import concourse.mybir as mybir
import concourse.tile as tile
from concourse import bacc
from concourse.bass import ds, ts
from concourse.bass_utils import run_bass_kernel_spmd

F32 = mybir.dt.float32
BF16 = mybir.dt.bfloat16
EXP = mybir.ActivationFunctionType.Exp
MULT = mybir.AluOpType.mult
ADD = mybir.AluOpType.add
BF = ml_dtypes.bfloat16

B, S, D, H, DH = 4, 2048, 1024, 16, 64
HALF = D // 2          # 512 douts per core
DT = HALF // 128       # 4 dout tiles
DIN = D // 128         # 8 din tiles
QB = S // 512          # 4 query blocks
KT = S // 128          # 16 key tiles / seq tiles

TRACE = False
LAST_EXEC_NS = None
LAST_TRACE = None
_NC = None


def _build():
    nc = bacc.Bacc("TRN2", target_bir_lowering=False, debug=False,
                   num_devices=8, name="mha")

    qT_d = nc.dram_tensor("qT", [D, S], BF16, kind="ExternalInput")
    kT_d = nc.dram_tensor("kT", [D, S], BF16, kind="ExternalInput")
    vT_d = nc.dram_tensor("vT", [D, S], BF16, kind="ExternalInput")
    wq_d = nc.dram_tensor("wq", [D, HALF], BF16, kind="ExternalInput")
    wk_d = nc.dram_tensor("wk", [D, HALF], BF16, kind="ExternalInput")
    wv_d = nc.dram_tensor("wv", [D, HALF], BF16, kind="ExternalInput")
    wo_d = nc.dram_tensor("wo", [HALF, D], BF16, kind="ExternalInput")
    bqc_d = nc.dram_tensor("bqc", [128, DT], F32, kind="ExternalInput")
    # bf16 output: host sums the two half partials in f32; the added ~0.3%
    # RMS is well inside the error budget and halves the output DMA bytes.
    out_d = nc.dram_tensor("out", [S, D], BF16, kind="ExternalOutput")

    kT_r = kT_d[:].rearrange("(o p) f -> o p f", p=128)
    vT_r = vT_d[:].rearrange("(o p) f -> o p f", p=128)
    qT_r = qT_d[:].rearrange("(o p) f -> o p f", p=128)

    stk = ExitStack()
    with tile.TileContext(nc) as tc:
        persist = stk.enter_context(tc.tile_pool(name="persist", bufs=1))
        xin = stk.enter_context(tc.tile_pool(name="xin", bufs=16))
        qch = stk.enter_context(tc.tile_pool(name="qch", bufs=16))
        pTp = stk.enter_context(tc.tile_pool(name="pTp", bufs=3))
        otsb = stk.enter_context(tc.tile_pool(name="otsb", bufs=3))
        nrm = stk.enter_context(tc.tile_pool(name="nrm", bufs=1))
        outsb = stk.enter_context(tc.tile_pool(name="outsb", bufs=2))
        ps_pair = stk.enter_context(tc.tile_pool(name="ps_pair", bufs=2, space="PSUM"))
        ps_ot = stk.enter_context(tc.tile_pool(name="ps_ot", bufs=2, space="PSUM"))
        ps_proj = stk.enter_context(tc.tile_pool(name="ps_proj", bufs=2, space="PSUM"))

        # --- persistent SBUF (all split per-(dt, block) to avoid false deps) ---
        wq_sb = persist.tile([128, DIN, HALF], BF16)
        wk_sb = persist.tile([128, DIN, HALF], BF16)
        wv_sb = persist.tile([128, DIN, HALF], BF16)
        wo_sb = persist.tile([128, DT, D], BF16)
        bq_sb = persist.tile([128, DT], F32)
        ones_col = persist.tile([1, 64], F32)
        nc.vector.memset(ones_col[:], 1.0)
        qPt = {}
        kPt = {}
        aTt = {}
        vat = {}
        for dt in range(DT):
            for qb in range(QB):
                qPt[(dt, qb)] = persist.tile([128, 512], BF16,
                                             name=f"qP_{dt}_{qb}")
                kPt[(dt, qb)] = persist.tile([128, 512], BF16,
                                             name=f"kP_{dt}_{qb}")
                aTt[(dt, qb)] = persist.tile([128, 512], BF16,
                                             name=f"aT_{dt}_{qb}")
        for st in range(KT):
            vat[st] = persist.tile([128, 8 * 65], BF16, name=f"va_{st}")
            nc.vector.memset(vat[st][:], 1.0)

        kin = [xin.tile([128, S], BF16, tag="xin", name=f"kin{d}")
               for d in range(DIN)]
        vin = [xin.tile([128, S], BF16, tag="xin", name=f"vin{d}")
               for d in range(DIN)]
        qchunks = {}

        def load_qchunks(qb, eng=None):
            for d in range(DIN):
                t = qch.tile([128, 512], BF16, tag="qch")
                (eng or nc.sync).dma_start(t[:], qT_r[d][:, ts(qb, 512)])
                qchunks[(d, qb)] = t

        # ---- input DMAs ----
        # Sync queue: the T0-critical stream (qb0 chunks first - qproj leads
        # the ramp - then kT).
        nc.sync.dma_start(wq_sb[:], wq_d[:].rearrange("(o p) n -> p o n", p=128))
        nc.sync.dma_start(bq_sb[:], bqc_d[:])
        load_qchunks(0)
        nc.sync.dma_start(wk_sb[:], wk_d[:].rearrange("(o p) n -> p o n", p=128))
        for d in range(DIN):
            nc.sync.dma_start(kin[d][:, ts(0, 512)], kT_r[d][:, ts(0, 512)])
        for blk in range(1, 4):
            for d in range(DIN):
                nc.sync.dma_start(kin[d][:, ts(blk, 512)], kT_r[d][:, ts(blk, 512)])
        nc.sync.dma_start(wo_sb[:], wo_d[:].rearrange("(o p) n -> p o n", p=128))
        # GpSimd queue: wv/vT, gated behind the last kin-b0 chunk so the vT
        # stream cannot steal bandwidth from the ramp-critical bytes.
        probe = persist.tile([1, 4], BF16)
        nc.gpsimd.tensor_copy(probe[:], kin[DIN - 1][0:1, 0:4])
        nc.gpsimd.dma_start(wv_sb[:], wv_d[:].rearrange("(o p) n -> p o n", p=128))
        for blk in range(4):
            for d in range(DIN):
                nc.gpsimd.dma_start(vin[d][:, ts(blk, 512)],
                                    vT_r[d][:, ts(blk, 512)])

        # ---- projection emitters ----
        def kproj(dt, qbk):
            ps = ps_proj.tile([128, 512], F32, tag="proj")
            for d in range(DIN):
                nc.tensor.matmul(ps[:], wk_sb[:, d, ts(dt, 128)],
                                 kin[d][:, ts(qbk, 512)],
                                 start=(d == 0), stop=(d == DIN - 1))
            nc.vector.tensor_copy(kPt[(dt, qbk)][:], ps[:])

        def qproj(dt, qb):
            ps = ps_proj.tile([128, 512], F32, tag="proj")
            for d in range(DIN):
                nc.tensor.matmul(ps[:], wq_sb[:, d, ts(dt, 128)],
                                 qchunks[(d, qb)][:],
                                 start=(d == 0), stop=(d == DIN - 1))
            nc.vector.tensor_scalar(qPt[(dt, qb)][:], ps[:],
                                    scalar1=bq_sb[:, dt:dt + 1], scalar2=None,
                                    op0=ADD)

        def proj_items(kind, dt, qb):
            """Split an 8-MM projection group into 2-MM drip-feed closures."""
            state = {}
            w_sb = wk_sb if kind == "k" else wq_sb

            def src(d):
                if kind == "k":
                    return kin[d][:, ts(qb, 512)]
                return qchunks[(d, qb)][:]

            def mk(d0):
                def mm():
                    if d0 == 0:
                        state["ps"] = ps_proj.tile([128, 512], F32, tag="proj",
                                                   name="proj_ps")
                    ps = state["ps"]
                    for d in (d0, d0 + 1):
                        nc.tensor.matmul(ps[:], w_sb[:, d, ts(dt, 128)],
                                         src(d),
                                         start=(d == 0), stop=(d == DIN - 1))
                return mm

            def wb():
                if kind == "k":
                    nc.vector.tensor_copy(kPt[(dt, qb)][:], state["ps"][:])
                else:
                    nc.vector.tensor_scalar(qPt[(dt, qb)][:], state["ps"][:],
                                            scalar1=bq_sb[:, dt:dt + 1],
                                            scalar2=None, op0=ADD)

            return [mk(0), mk(2), mk(4), mk(6), wb]

        def v_proj(st):
            ps = ps_proj.tile([128, 512], F32, tag="proj")
            for d in range(DIN):
                nc.tensor.matmul(ps[:], vin[d][:, ts(st, 128)], wv_sb[:, d, :],
                                 start=(d == 0), stop=(d == DIN - 1))
            nc.vector.tensor_copy(
                vat[st][:].rearrange("p (h c) -> p h c", h=8)[:, :, 0:64],
                ps[:].rearrange("p (h c) -> p h c", h=8))

        def outproj_items(qb):
            items = []
            for j in range(4):
                st = qb * 4 + j
                for half in range(2):
                    state = {}

                    def mk(st=st, half=half, state=state):
                        jq, jl = st // 4, st % 4

                        def mm_a():
                            ps = ps_proj.tile([128, 512], F32, tag="proj")
                            state["ps"] = ps
                            for dt in (0, 1):
                                nc.tensor.matmul(ps[:],
                                                 aTt[(dt, jq)][:, ts(jl, 128)],
                                                 wo_sb[:, dt, ts(half, 512)],
                                                 start=(dt == 0), stop=False)

                        def mm_b():
                            ps = state["ps"]
                            for dt in (2, 3):
                                nc.tensor.matmul(ps[:],
                                                 aTt[(dt, jq)][:, ts(jl, 128)],
                                                 wo_sb[:, dt, ts(half, 512)],
                                                 start=False, stop=(dt == 3))

                        def wb():
                            ps = state["ps"]
                            osb = outsb.tile([128, 512], BF16, tag="osb")
                            nc.vector.tensor_copy(osb[:], ps[:])
                            nc.sync.dma_start(
                                out_d[ds(st * 128, 128), ts(half, 512)], osb[:])

                        return [mm_a, mm_b, wb]

                    items += mk()
            return items

        # ---- ramp: just enough to start (qb0, hp0) ----
        qproj(0, 0)
        kproj(0, 0)

        # ---- filler schedules per combo ----
        # kproj(dt, qbk) must land before combo (qb0, hp=dt) reaches step
        # 4*qbk; qproj(dt, qb) before combo (qb, hp=dt).
        PROJ_SCHED = {
            0: [("k", 0, 1), ("k", 0, 2), ("q", 1, 0), ("k", 0, 3),
                ("q", 2, 0), ("k", 1, 0), ("q", 3, 0)],
            1: [("k", 1, 1), ("k", 1, 2), ("k", 1, 3), ("k", 2, 0)],
            2: [("k", 2, 1), ("k", 2, 2), ("k", 2, 3), ("k", 3, 0)],
            3: [("k", 3, 1), ("k", 3, 2), ("k", 3, 3), ("q", 0, 1)],
            4: [("q", 1, 1), ("q", 2, 1)],
            5: [("q", 3, 1)], 6: [("q", 0, 2)], 7: [("q", 1, 2)],
            8: [("q", 2, 2)], 9: [("q", 3, 2)], 10: [("q", 0, 3)],
            11: [("q", 1, 3)], 12: [("q", 2, 3)], 13: [("q", 3, 3)],
        }

        def build_fillers(ci, qb, hp):
            f, late = [], []
            for kind, dt, qbx in PROJ_SCHED.get(ci, []):
                f += proj_items(kind, dt, qbx)
            if qb > 0:
                late = outproj_items(qb - 1)[hp * 6:(hp + 1) * 6]
            return f, late

        # ---- attention: software-pipelined over all 16 combos ----
        combos = [(qb, hp) for qb in range(QB) for hp in range(DT)]
        pending_tail = None

        def make_tail(otA, otB, last_p, hp, qb):
            def tail():
                pkt, pp = last_p
                nc.tensor.matmul(otA[0:65, :], vat[pkt][:, ds(2 * hp * 65, 65)],
                                 pp[:, 0:512], start=False, stop=True)
                nc.tensor.matmul(otB[0:65, :],
                                 vat[pkt][:, ds((2 * hp + 1) * 65, 65)],
                                 pp[:, 512:1024], start=False, stop=True)
                oa = otsb.tile([128, 512], F32, tag="ot_sb")
                ob = otsb.tile([128, 512], F32, tag="ot_sb")
                nc.vector.tensor_copy(oa[0:64, :], otA[0:64, :])
                nc.vector.tensor_copy(ob[0:64, :], otB[0:64, :])
                sm = nrm.tile([1, 1024], F32, tag="sums")
                nc.vector.tensor_copy(sm[0:1, 0:512], otA[64:65, :])
                nc.vector.tensor_copy(sm[0:1, 512:1024], otB[64:65, :])
                r = nrm.tile([1, 1024], F32, tag="recip")
                nc.vector.reciprocal_approx_fast(r[0:1, :], sm[0:1, :])
                rb = nrm.tile([64, 1024], F32, tag="rb")
                nc.gpsimd.partition_broadcast(rb[:], r[0:1, :])
                nc.vector.tensor_tensor(aTt[(hp, qb)][0:64, :],
                                        oa[0:64, :], rb[:, 0:512], MULT)
                nc.vector.tensor_tensor(aTt[(hp, qb)][64:128, :],
                                        ob[0:64, :], rb[:, 512:1024], MULT)
            return tail

        for ci, (qb, hp) in enumerate(combos):
            if hp == 0 and qb < QB - 1:
                load_qchunks(qb + 1, eng=nc.gpsimd)
            early_f, late_f = build_fillers(ci, qb, hp)
            fillers = deque(early_f)
            late = deque(late_f)
            otA = otB = None
            prev_p = None
            for kt in range(KT):
                pair = ps_pair.tile([128, 1024], F32, tag="pair")
                nc.tensor.matmul(pair[:, 0:512],
                                 kPt[(hp, kt // 4)][0:64, ts(kt % 4, 128)],
                                 qPt[(hp, qb)][0:64, :],
                                 start=True, stop=True, tile_position=(0, 0))
                nc.tensor.matmul(pair[:, 512:1024],
                                 kPt[(hp, kt // 4)][64:128, ts(kt % 4, 128)],
                                 qPt[(hp, qb)][64:128, :],
                                 start=True, stop=True, tile_position=(64, 0))
                p = pTp.tile([128, 1024], BF16, tag="pT")
                nc.scalar.activation(p[:], pair[:], EXP, scale=0.125)
                if kt == 0:
                    if pending_tail is not None:
                        pending_tail()
                        pending_tail = None
                    # alloc AFTER the previous combo's tail is emitted so the
                    # pool's WAR deps order this combo's AV writes correctly
                    otA = ps_ot.tile([128, 512], F32, tag="ot")
                    otB = ps_ot.tile([128, 512], F32, tag="ot")
                if qb == 0 and hp == 0:
                    v_proj(kt)
                if prev_p is not None:
                    pkt, pp = prev_p
                    nc.tensor.matmul(otA[0:65, :],
                                     vat[pkt][:, ds(2 * hp * 65, 65)],
                                     pp[:, 0:512],
                                     start=(pkt == 0), stop=False)
                    nc.tensor.matmul(otB[0:65, :],
                                     vat[pkt][:, ds((2 * hp + 1) * 65, 65)],
                                     pp[:, 512:1024],
                                     start=(pkt == 0), stop=False)
                prev_p = (kt, p)
                # proj fillers pop from kt>=2 on qb>0 combos; outproj ("late")
                # fillers only from kt>=8, so they never chase the previous
                # combo's still-in-flight attnT write (the tail's DVE+GpSimd
                # chain lands the TT around kt6.5).
                if not (qb > 0 and kt < 2):
                    avail = len(fillers) + (len(late) if kt >= 8 else 0)
                    steps_left = KT - kt
                    want = -(-(len(fillers) + len(late)) // steps_left)
                    pops = min(avail, max(1, want))
                    for _ in range(pops):
                        (fillers if fillers else late).popleft()()
            while fillers:
                fillers.popleft()()
            while late:
                late.popleft()()
            if ci < len(combos) - 1:
                pending_tail = make_tail(otA, otB, prev_p, hp, qb)

        # ---- final tail, latency-ordered ----
        # The last combo's normalize chain gates the qb3 out-projection, so:
        # final AVs -> sums (DVE) -> pre-open FOUR outproj groups' dt0-2
        # partials (~3us of PE streaming keeps HAM warm through the chain)
        # -> recip + GpSimd broadcast with oa/ob drains overlapped -> TT
        # normalize -> dt3 closers + remaining groups, all at warm speed.
        hp, qb = combos[-1]
        pkt, pp = prev_p
        nc.tensor.matmul(otA[0:65, :], vat[pkt][:, ds(2 * hp * 65, 65)],
                         pp[:, 0:512], start=False, stop=True)
        nc.tensor.matmul(otB[0:65, :], vat[pkt][:, ds((2 * hp + 1) * 65, 65)],
                         pp[:, 512:1024], start=False, stop=True)
        sm = nrm.tile([1, 1024], F32, tag="sums")
        nc.vector.tensor_copy(sm[0:1, 0:512], otA[64:65, :])
        nc.vector.tensor_copy(sm[0:1, 512:1024], otB[64:65, :])
        st0 = (QB - 1) * 4
        pre = {}
        for half in range(2):
            tps = ps_proj.tile([128, 512], F32, tag="proj", name="tail_pre")
            pre[(st0, half)] = tps[:]
            for dt in (0, 1, 2):
                nc.tensor.matmul(tps[:], aTt[(dt, QB - 1)][:, ts(0, 128)],
                                 wo_sb[:, dt, ts(half, 512)],
                                 start=(dt == 0), stop=False)
        pre2 = ps_pair.tile([128, 1024], F32, tag="pair")
        for half in range(2):
            pre[(st0 + 1, half)] = pre2[:, ts(half, 512)]
            for dt in (0, 1, 2):
                nc.tensor.matmul(pre2[:, ts(half, 512)],
                                 aTt[(dt, QB - 1)][:, ts(1, 128)],
                                 wo_sb[:, dt, ts(half, 512)],
                                 start=(dt == 0), stop=False)
        r = nrm.tile([1, 1024], F32, tag="recip")
        nc.vector.reciprocal_approx_fast(r[0:1, :], sm[0:1, :])
        rb = nrm.tile([64, 1024], F32, tag="rb")
        nc.gpsimd.partition_broadcast(rb[:], r[0:1, :])
        oa = otsb.tile([128, 512], F32, tag="ot_sb")
        ob = otsb.tile([128, 512], F32, tag="ot_sb")
        nc.vector.tensor_copy(oa[0:64, :], otA[0:64, :])
        nc.vector.tensor_copy(ob[0:64, :], otB[0:64, :])
        # normalize in 128-col pieces so each st's dt3 closer can start as
        # soon as its slice of aTt[(3,3)] lands instead of after both full TTs
        for j in range(4):
            nc.vector.tensor_tensor(aTt[(hp, qb)][0:64, ts(j, 128)],
                                    oa[0:64, ts(j, 128)],
                                    rb[:, ds(j * 128, 128)], MULT)
            nc.vector.tensor_tensor(aTt[(hp, qb)][64:128, ts(j, 128)],
                                    ob[0:64, ts(j, 128)],
                                    rb[:, ds(512 + j * 128, 128)], MULT)
        for j in range(4):
            st = st0 + j
            for half in range(2):
                if (st, half) in pre:
                    pap = pre[(st, half)]
                    nc.tensor.matmul(pap, aTt[(3, QB - 1)][:, ts(j, 128)],
                                     wo_sb[:, 3, ts(half, 512)],
                                     start=False, stop=True)
                else:
                    tps = ps_proj.tile([128, 512], F32, tag="proj",
                                       name="tail_grp")
                    pap = tps[:]
                    for dt in range(4):
                        nc.tensor.matmul(pap, aTt[(dt, QB - 1)][:, ts(j, 128)],
                                         wo_sb[:, dt, ts(half, 512)],
                                         start=(dt == 0), stop=(dt == 3))
                osb = outsb.tile([128, 512], BF16, tag="osb")
                nc.vector.tensor_copy(osb[:], pap)
                nc.sync.dma_start(out_d[ds(st * 128, 128), ts(half, 512)],
                                  osb[:])

        stk.close()

    nc.finalize()
    return nc


def kernel(q, k, v, mask, w_q, b_q, w_k, b_k, w_v, b_v, w_o, b_o):
    global _NC, LAST_EXEC_NS, LAST_TRACE
    if _NC is None:
        _NC = _build()
    nc = _NC

    q = np.asarray(q, np.float32)
    k = np.asarray(k, np.float32)
    v = np.asarray(v, np.float32)
    w_q = np.asarray(w_q, np.float32)
    w_k = np.asarray(w_k, np.float32)
    w_v = np.asarray(w_v, np.float32)
    w_o = np.asarray(w_o, np.float32)
    b_q = np.asarray(b_q, np.float32)
    b_v = np.asarray(b_v, np.float32)
    b_o = np.asarray(b_o, np.float32)

    in_maps = []
    for c in range(8):
        b, hf = divmod(c, 2)
        sl = slice(hf * HALF, (hf + 1) * HALF)
        in_maps.append({
            "qT": q[b].T.astype(BF),
            "kT": k[b].T.astype(BF),
            "vT": v[b].T.astype(BF),
            "wq": w_q[sl, :].T.astype(BF),
            "wk": w_k[sl, :].T.astype(BF),
            "wv": w_v[sl, :].T.astype(BF),
            "wo": w_o[:, sl].T.astype(BF),
            "bqc": b_q[sl].reshape(DT, 128).T.copy(),
        })

    kwargs = {}
    if TRACE:
        kwargs = dict(trace=True, trace_cores=[0])
    try:
        res = run_bass_kernel_spmd(nc, in_maps, core_ids=list(range(8)), **kwargs)
    except Exception:
        # transient device wedge (e.g. a previously killed client left a core
        # dirty) usually clears on retry
        time.sleep(2.0)
        res = run_bass_kernel_spmd(nc, in_maps, core_ids=list(range(8)), **kwargs)
    if TRACE:
        LAST_EXEC_NS = res.exec_time_ns
        LAST_TRACE = res.instructions_and_trace[1] if res.instructions_and_trace else None

    # b_v folds to a constant through the attention (rows sum to 1) and the
    # row-parallel out-projection: add b_v @ w_o.T once on the host.
    const = b_o + b_v @ w_o.T
    out = np.empty((B, S, D), np.float32)
    for b in range(B):
        out[b] = (res.results[2 * b]["out"].astype(np.float32)
                  + res.results[2 * b + 1]["out"].astype(np.float32)
                  + const[None, :])
    return out


# revision 36
# speedup vs baseline: 1.0080x; 1.0080x over previous
"""Multi-head attention (B=4, S=2048, D=1024, H=16) on 8 Trainium2 cores.

Sharding: data parallel on batch (4) x tensor parallel on heads (2 halves of
8 heads). Core c handles batch c//2 and head-half c%2: column-parallel
w_q/w_k/w_v (512 out dims), local attention over its 8 heads, row-parallel
w_o (its 512 hd columns) producing a full [2048, 1024] partial that the host
sums across the two halves (plus b_o and the folded b_v @ w_o.T term).

On-device layout is feature-on-partitions throughout ("transposed"):
  qP/kP: per-(dt, block) [128, 512] bf16 tiles (projection form B)
  scores S.T: [keys, queries] via paired K=64 matmuls (head pair at PE row
  offsets 0/64 with tile_position) into a 2-bank PSUM tile, one wide exp ACT
  AV: O.T accumulation with V_aug ones-column producing row sums; normalize
  via DVE fast reciprocal + GpSimd partition-broadcast.

Bias handling (no bias matmuls at all):
  b_k drops: q.b_k is constant per query -> softmax-invariant.
  b_q folds into the Q writeback as a DVE per-partition tensor_scalar add.
  b_v folds into the host-side add: attn rows sum to 1, so its effect is
  the constant vector b_v @ w_o.T.

All persistent tensors are split into per-(dt, block) [128, 512] tiles so
the Tile dep tracker never sees false write-read sharing between disjoint
slices (a single big tile serializes later readers on the freshest write).

Software pipelined schedule: steady state is one exp ACT per kt step (ACT
~100% busy); each combo's first score pair + exp are emitted BEFORE the
previous combo's final-AV/normalize tail so ACT never idles at boundaries.
Input DMAs are column-chunked: the Sync queue carries the T0-critical
wk/kT/wq/qT-qb0 stream, the GpSimd queue carries wv/vT gated behind a probe
copy of the last qb0 chunk so it cannot steal bandwidth from the ramp.
"""

import time
from collections import deque
from contextlib import ExitStack

import ml_dtypes
import numpy as np

import concourse.bass as bass
import concourse.mybir as mybir
import concourse.tile as tile
from concourse import bacc
from concourse.bass import ds, ts
from concourse.bass_utils import run_bass_kernel_spmd

F32 = mybir.dt.float32
BF16 = mybir.dt.bfloat16
EXP = mybir.ActivationFunctionType.Exp
MULT = mybir.AluOpType.mult
ADD = mybir.AluOpType.add
BF = ml_dtypes.bfloat16

B, S, D, H, DH = 4, 2048, 1024, 16, 64
HALF = D // 2          # 512 douts per core
DT = HALF // 128       # 4 dout tiles
DIN = D // 128         # 8 din tiles
QB = S // 512          # 4 query blocks
KT = S // 128          # 16 key tiles / seq tiles

TRACE = False
LAST_EXEC_NS = None
LAST_TRACE = None
_NC = None


def _build():
    nc = bacc.Bacc("TRN2", target_bir_lowering=False, debug=False,
                   num_devices=8, name="mha")

    qT_d = nc.dram_tensor("qT", [D, S], BF16, kind="ExternalInput")
    kT_d = nc.dram_tensor("kT", [D, S], BF16, kind="ExternalInput")
    vT_d = nc.dram_tensor("vT", [D, S], BF16, kind="ExternalInput")
    wq_d = nc.dram_tensor("wq", [D, HALF], BF16, kind="ExternalInput")
    wk_d = nc.dram_tensor("wk", [D, HALF], BF16, kind="ExternalInput")
    wv_d = nc.dram_tensor("wv", [D, HALF], BF16, kind="ExternalInput")
    wo_d = nc.dram_tensor("wo", [HALF, D], BF16, kind="ExternalInput")
    bqc_d = nc.dram_tensor("bqc", [128, DT], F32, kind="ExternalInput")
    # bf16 output: host sums the two half partials in f32; the added ~0.3%
    # RMS is well inside the error budget and halves the output DMA bytes.
    out_d = nc.dram_tensor("out", [S, D], BF16, kind="ExternalOutput")

    kT_r = kT_d[:].rearrange("(o p) f -> o p f", p=128)
    vT_r = vT_d[:].rearrange("(o p) f -> o p f", p=128)
    qT_r = qT_d[:].rearrange("(o p) f -> o p f", p=128)

    stk = ExitStack()
    with tile.TileContext(nc) as tc:
        persist = stk.enter_context(tc.tile_pool(name="persist", bufs=1))
        xin = stk.enter_context(tc.tile_pool(name="xin", bufs=16))
        qch = stk.enter_context(tc.tile_pool(name="qch", bufs=16))
        pTp = stk.enter_context(tc.tile_pool(name="pTp", bufs=3))
        otsb = stk.enter_context(tc.tile_pool(name="otsb", bufs=3))
        nrm = stk.enter_context(tc.tile_pool(name="nrm", bufs=1))
        outsb = stk.enter_context(tc.tile_pool(name="outsb", bufs=2))
        ps_pair = stk.enter_context(tc.tile_pool(name="ps_pair", bufs=2, space="PSUM"))
        ps_ot = stk.enter_context(tc.tile_pool(name="ps_ot", bufs=2, space="PSUM"))
        ps_proj = stk.enter_context(tc.tile_pool(name="ps_proj", bufs=2, space="PSUM"))

        # --- persistent SBUF (all split per-(dt, block) to avoid false deps) ---
        wq_sb = persist.tile([128, DIN, HALF], BF16)
        wk_sb = persist.tile([128, DIN, HALF], BF16)
        wv_sb = persist.tile([128, DIN, HALF], BF16)
        wo_sb = persist.tile([128, DT, D], BF16)
        bq_sb = persist.tile([128, DT], F32)
        ones_col = persist.tile([1, 64], F32)
        nc.vector.memset(ones_col[:], 1.0)
        qPt = {}
        kPt = {}
        aTt = {}
        vat = {}
        for dt in range(DT):
            for qb in range(QB):
                qPt[(dt, qb)] = persist.tile([128, 512], BF16,
                                             name=f"qP_{dt}_{qb}")
                kPt[(dt, qb)] = persist.tile([128, 512], BF16,
                                             name=f"kP_{dt}_{qb}")
                aTt[(dt, qb)] = persist.tile([128, 512], BF16,
                                             name=f"aT_{dt}_{qb}")
        for st in range(KT):
            vat[st] = persist.tile([128, 8 * 65], BF16, name=f"va_{st}")
            nc.vector.memset(vat[st][:], 1.0)

        kin = [xin.tile([128, S], BF16, tag="xin", name=f"kin{d}")
               for d in range(DIN)]
        vin = [xin.tile([128, S], BF16, tag="xin", name=f"vin{d}")
               for d in range(DIN)]
        qchunks = {}

        def load_qchunks(qb, eng=None):
            for d in range(DIN):
                t = qch.tile([128, 512], BF16, tag="qch")
                (eng or nc.sync).dma_start(t[:], qT_r[d][:, ts(qb, 512)])
                qchunks[(d, qb)] = t

        # ---- input DMAs ----
        # Sync queue: the T0-critical stream (qb0 chunks first - qproj leads
        # the ramp - then kT).
        nc.sync.dma_start(wk_sb[:], wk_d[:].rearrange("(o p) n -> p o n", p=128))
        nc.sync.dma_start(wq_sb[:], wq_d[:].rearrange("(o p) n -> p o n", p=128))
        nc.sync.dma_start(bq_sb[:], bqc_d[:])
        load_qchunks(0)
        for d in range(DIN):
            nc.sync.dma_start(kin[d][:, ts(0, 512)], kT_r[d][:, ts(0, 512)])
        for blk in range(1, 4):
            for d in range(DIN):
                nc.sync.dma_start(kin[d][:, ts(blk, 512)], kT_r[d][:, ts(blk, 512)])
        nc.sync.dma_start(wo_sb[:], wo_d[:].rearrange("(o p) n -> p o n", p=128))
        # GpSimd queue: wv/vT, gated behind the last kin-b0 chunk so the vT
        # stream cannot steal bandwidth from the ramp-critical bytes.
        probe = persist.tile([1, 4], BF16)
        nc.gpsimd.tensor_copy(probe[:], kin[DIN - 1][0:1, 0:4])
        nc.gpsimd.dma_start(wv_sb[:], wv_d[:].rearrange("(o p) n -> p o n", p=128))
        for blk in range(4):
            for d in range(DIN):
                nc.gpsimd.dma_start(vin[d][:, ts(blk, 512)],
                                    vT_r[d][:, ts(blk, 512)])

        # ---- projection emitters ----
        def kproj(dt, qbk):
            ps = ps_proj.tile([128, 512], F32, tag="proj")
            for d in range(DIN):
                nc.tensor.matmul(ps[:], wk_sb[:, d, ts(dt, 128)],
                                 kin[d][:, ts(qbk, 512)],
                                 start=(d == 0), stop=(d == DIN - 1))
            nc.vector.tensor_copy(kPt[(dt, qbk)][:], ps[:])

        def qproj(dt, qb):
            ps = ps_proj.tile([128, 512], F32, tag="proj")
            for d in range(DIN):
                nc.tensor.matmul(ps[:], wq_sb[:, d, ts(dt, 128)],
                                 qchunks[(d, qb)][:],
                                 start=(d == 0), stop=(d == DIN - 1))
            nc.vector.tensor_scalar(qPt[(dt, qb)][:], ps[:],
                                    scalar1=bq_sb[:, dt:dt + 1], scalar2=None,
                                    op0=ADD)

        def proj_items(kind, dt, qb):
            """Split an 8-MM projection group into 2-MM drip-feed closures."""
            state = {}
            w_sb = wk_sb if kind == "k" else wq_sb

            def src(d):
                if kind == "k":
                    return kin[d][:, ts(qb, 512)]
                return qchunks[(d, qb)][:]

            def mk(d0):
                def mm():
                    if d0 == 0:
                        state["ps"] = ps_proj.tile([128, 512], F32, tag="proj",
                                                   name="proj_ps")
                    ps = state["ps"]
                    for d in (d0, d0 + 1):
                        nc.tensor.matmul(ps[:], w_sb[:, d, ts(dt, 128)],
                                         src(d),
                                         start=(d == 0), stop=(d == DIN - 1))
                return mm

            def wb():
                if kind == "k":
                    nc.vector.tensor_copy(kPt[(dt, qb)][:], state["ps"][:])
                else:
                    nc.vector.tensor_scalar(qPt[(dt, qb)][:], state["ps"][:],
                                            scalar1=bq_sb[:, dt:dt + 1],
                                            scalar2=None, op0=ADD)

            return [mk(0), mk(2), mk(4), mk(6), wb]

        def v_proj(st):
            ps = ps_proj.tile([128, 512], F32, tag="proj")
            for d in range(DIN):
                nc.tensor.matmul(ps[:], vin[d][:, ts(st, 128)], wv_sb[:, d, :],
                                 start=(d == 0), stop=(d == DIN - 1))
            nc.vector.tensor_copy(
                vat[st][:].rearrange("p (h c) -> p h c", h=8)[:, :, 0:64],
                ps[:].rearrange("p (h c) -> p h c", h=8))

        def outproj_items(qb):
            items = []
            for j in range(4):
                st = qb * 4 + j
                for half in range(2):
                    state = {}

                    def mk(st=st, half=half, state=state):
                        jq, jl = st // 4, st % 4

                        def mm_a():
                            ps = ps_proj.tile([128, 512], F32, tag="proj")
                            state["ps"] = ps
                            for dt in (0, 1):
                                nc.tensor.matmul(ps[:],
                                                 aTt[(dt, jq)][:, ts(jl, 128)],
                                                 wo_sb[:, dt, ts(half, 512)],
                                                 start=(dt == 0), stop=False)

                        def mm_b():
                            ps = state["ps"]
                            for dt in (2, 3):
                                nc.tensor.matmul(ps[:],
                                                 aTt[(dt, jq)][:, ts(jl, 128)],
                                                 wo_sb[:, dt, ts(half, 512)],
                                                 start=False, stop=(dt == 3))

                        def wb():
                            ps = state["ps"]
                            osb = outsb.tile([128, 512], BF16, tag="osb")
                            nc.vector.tensor_copy(osb[:], ps[:])
                            nc.sync.dma_start(
                                out_d[ds(st * 128, 128), ts(half, 512)], osb[:])

                        return [mm_a, mm_b, wb]

                    items += mk()
            return items

        # ---- ramp: just enough to start (qb0, hp0) ----
        qproj(0, 0)
        kproj(0, 0)

        # ---- filler schedules per combo ----
        # kproj(dt, qbk) must land before combo (qb0, hp=dt) reaches step
        # 4*qbk; qproj(dt, qb) before combo (qb, hp=dt).
        PROJ_SCHED = {
            0: [("k", 0, 1), ("k", 0, 2), ("q", 1, 0), ("k", 0, 3),
                ("q", 2, 0), ("k", 1, 0), ("q", 3, 0)],
            1: [("k", 1, 1), ("k", 1, 2), ("k", 1, 3), ("k", 2, 0)],
            2: [("k", 2, 1), ("k", 2, 2), ("k", 2, 3), ("k", 3, 0)],
            3: [("k", 3, 1), ("k", 3, 2), ("k", 3, 3), ("q", 0, 1)],
            4: [("q", 1, 1), ("q", 2, 1)],
            5: [("q", 3, 1)], 6: [("q", 0, 2)], 7: [("q", 1, 2)],
            8: [("q", 2, 2)], 9: [("q", 3, 2)], 10: [("q", 0, 3)],
            11: [("q", 1, 3)], 12: [("q", 2, 3)], 13: [("q", 3, 3)],
        }

        def build_fillers(ci, qb, hp):
            f, late = [], []
            for kind, dt, qbx in PROJ_SCHED.get(ci, []):
                f += proj_items(kind, dt, qbx)
            if qb > 0:
                late = outproj_items(qb - 1)[hp * 6:(hp + 1) * 6]
            return f, late

        # ---- attention: software-pipelined over all 16 combos ----
        combos = [(qb, hp) for qb in range(QB) for hp in range(DT)]
        pending_tail = None

        def make_tail(otA, otB, last_p, hp, qb):
            def tail():
                pkt, pp = last_p
                nc.tensor.matmul(otA[0:65, :], vat[pkt][:, ds(2 * hp * 65, 65)],
                                 pp[:, 0:512], start=False, stop=True)
                nc.tensor.matmul(otB[0:65, :],
                                 vat[pkt][:, ds((2 * hp + 1) * 65, 65)],
                                 pp[:, 512:1024], start=False, stop=True)
                oa = otsb.tile([128, 512], F32, tag="ot_sb")
                ob = otsb.tile([128, 512], F32, tag="ot_sb")
                nc.vector.tensor_copy(oa[0:64, :], otA[0:64, :])
                nc.vector.tensor_copy(ob[0:64, :], otB[0:64, :])
                sm = nrm.tile([1, 1024], F32, tag="sums")
                nc.vector.tensor_copy(sm[0:1, 0:512], otA[64:65, :])
                nc.vector.tensor_copy(sm[0:1, 512:1024], otB[64:65, :])
                r = nrm.tile([1, 1024], F32, tag="recip")
                nc.vector.reciprocal_approx_fast(r[0:1, :], sm[0:1, :])
                rb = nrm.tile([64, 1024], F32, tag="rb")
                nc.gpsimd.partition_broadcast(rb[:], r[0:1, :])
                nc.vector.tensor_tensor(aTt[(hp, qb)][0:64, :],
                                        oa[0:64, :], rb[:, 0:512], MULT)
                nc.vector.tensor_tensor(aTt[(hp, qb)][64:128, :],
                                        ob[0:64, :], rb[:, 512:1024], MULT)
            return tail

        for ci, (qb, hp) in enumerate(combos):
            if hp == 0 and qb < QB - 1:
                load_qchunks(qb + 1, eng=nc.gpsimd)
            early_f, late_f = build_fillers(ci, qb, hp)
            fillers = deque(early_f)
            late = deque(late_f)
            otA = otB = None
            prev_p = None
            for kt in range(KT):
                pair = ps_pair.tile([128, 1024], F32, tag="pair")
                nc.tensor.matmul(pair[:, 0:512],
                                 kPt[(hp, kt // 4)][0:64, ts(kt % 4, 128)],
                                 qPt[(hp, qb)][0:64, :],
                                 start=True, stop=True, tile_position=(0, 0))
                nc.tensor.matmul(pair[:, 512:1024],
                                 kPt[(hp, kt // 4)][64:128, ts(kt % 4, 128)],
                                 qPt[(hp, qb)][64:128, :],
                                 start=True, stop=True, tile_position=(64, 0))
                p = pTp.tile([128, 1024], BF16, tag="pT")
                nc.scalar.activation(p[:], pair[:], EXP, scale=0.125)
                if kt == 0:
                    if pending_tail is not None:
                        pending_tail()
                        pending_tail = None
                    # alloc AFTER the previous combo's tail is emitted so the
                    # pool's WAR deps order this combo's AV writes correctly
                    otA = ps_ot.tile([128, 512], F32, tag="ot")
                    otB = ps_ot.tile([128, 512], F32, tag="ot")
                if qb == 0 and hp == 0:
                    v_proj(kt)
                if prev_p is not None:
                    pkt, pp = prev_p
                    nc.tensor.matmul(otA[0:65, :],
                                     vat[pkt][:, ds(2 * hp * 65, 65)],
                                     pp[:, 0:512],
                                     start=(pkt == 0), stop=False)
                    nc.tensor.matmul(otB[0:65, :],
                                     vat[pkt][:, ds((2 * hp + 1) * 65, 65)],
                                     pp[:, 512:1024],
                                     start=(pkt == 0), stop=False)
                prev_p = (kt, p)
                # proj fillers pop from kt>=2 on qb>0 combos; outproj ("late")
                # fillers only from kt>=6, so they never chase the previous
                # combo's still-in-flight attnT write.
                if not (qb > 0 and kt < 2):
                    avail = len(fillers) + (len(late) if kt >= 6 else 0)
                    steps_left = KT - kt
                    want = -(-(len(fillers) + len(late)) // steps_left)
                    pops = min(avail, max(1, want))
                    for _ in range(pops):
                        (fillers if fillers else late).popleft()()
            while fillers:
                fillers.popleft()()
            while late:
                late.popleft()()
            if ci < len(combos) - 1:
                pending_tail = make_tail(otA, otB, prev_p, hp, qb)

        # ---- final tail, latency-ordered ----
        # The last combo's normalize chain gates the qb3 out-projection, so:
        # final AVs -> sums (DVE) -> pre-open FOUR outproj groups' dt0-2
        # partials (~3us of PE streaming keeps HAM warm through the chain)
        # -> recip + GpSimd broadcast with oa/ob drains overlapped -> TT
        # normalize -> dt3 closers + remaining groups, all at warm speed.
        hp, qb = combos[-1]
        pkt, pp = prev_p
        nc.tensor.matmul(otA[0:65, :], vat[pkt][:, ds(2 * hp * 65, 65)],
                         pp[:, 0:512], start=False, stop=True)
        nc.tensor.matmul(otB[0:65, :], vat[pkt][:, ds((2 * hp + 1) * 65, 65)],
                         pp[:, 512:1024], start=False, stop=True)
        sm = nrm.tile([1, 1024], F32, tag="sums")
        nc.vector.tensor_copy(sm[0:1, 0:512], otA[64:65, :])
        nc.vector.tensor_copy(sm[0:1, 512:1024], otB[64:65, :])
        st0 = (QB - 1) * 4
        pre = {}
        for half in range(2):
            tps = ps_proj.tile([128, 512], F32, tag="proj", name="tail_pre")
            pre[(st0, half)] = tps[:]
            for dt in (0, 1, 2):
                nc.tensor.matmul(tps[:], aTt[(dt, QB - 1)][:, ts(0, 128)],
                                 wo_sb[:, dt, ts(half, 512)],
                                 start=(dt == 0), stop=False)
        pre2 = ps_pair.tile([128, 1024], F32, tag="pair")
        for half in range(2):
            pre[(st0 + 1, half)] = pre2[:, ts(half, 512)]
            for dt in (0, 1, 2):
                nc.tensor.matmul(pre2[:, ts(half, 512)],
                                 aTt[(dt, QB - 1)][:, ts(1, 128)],
                                 wo_sb[:, dt, ts(half, 512)],
                                 start=(dt == 0), stop=False)
        r = nrm.tile([1, 1024], F32, tag="recip")
        nc.vector.reciprocal_approx_fast(r[0:1, :], sm[0:1, :])
        rb = nrm.tile([64, 1024], F32, tag="rb")
        nc.gpsimd.partition_broadcast(rb[:], r[0:1, :])
        oa = otsb.tile([128, 512], F32, tag="ot_sb")
        ob = otsb.tile([128, 512], F32, tag="ot_sb")
        nc.vector.tensor_copy(oa[0:64, :], otA[0:64, :])
        nc.vector.tensor_copy(ob[0:64, :], otB[0:64, :])
        # normalize in 128-col pieces so each st's dt3 closer can start as
        # soon as its slice of aTt[(3,3)] lands instead of after both full TTs
        for j in range(4):
            nc.vector.tensor_tensor(aTt[(hp, qb)][0:64, ts(j, 128)],
                                    oa[0:64, ts(j, 128)],
                                    rb[:, ds(j * 128, 128)], MULT)
            nc.vector.tensor_tensor(aTt[(hp, qb)][64:128, ts(j, 128)],
                                    ob[0:64, ts(j, 128)],
                                    rb[:, ds(512 + j * 128, 128)], MULT)
        for j in range(4):
            st = st0 + j
            for half in range(2):
                if (st, half) in pre:
                    pap = pre[(st, half)]
                    nc.tensor.matmul(pap, aTt[(3, QB - 1)][:, ts(j, 128)],
                                     wo_sb[:, 3, ts(half, 512)],
                                     start=False, stop=True)
                else:
                    tps = ps_proj.tile([128, 512], F32, tag="proj",
                                       name="tail_grp")
                    pap = tps[:]
                    for dt in range(4):
                        nc.tensor.matmul(pap, aTt[(dt, QB - 1)][:, ts(j, 128)],
                                         wo_sb[:, dt, ts(half, 512)],
                                         start=(dt == 0), stop=(dt == 3))
                osb = outsb.tile([128, 512], BF16, tag="osb")
                nc.vector.tensor_copy(osb[:], pap)
                nc.sync.dma_start(out_d[ds(st * 128, 128), ts(half, 512)],
                                  osb[:])

        stk.close()

    nc.finalize()
    return nc


def kernel(q, k, v, mask, w_q, b_q, w_k, b_k, w_v, b_v, w_o, b_o):
    global _NC, LAST_EXEC_NS, LAST_TRACE
    if _NC is None:
        _NC = _build()
    nc = _NC

    q = np.asarray(q, np.float32)
    k = np.asarray(k, np.float32)
    v = np.asarray(v, np.float32)
    w_q = np.asarray(w_q, np.float32)
    w_k = np.asarray(w_k, np.float32)
    w_v = np.asarray(w_v, np.float32)
    w_o = np.asarray(w_o, np.float32)
    b_q = np.asarray(b_q, np.float32)
    b_v = np.asarray(b_v, np.float32)
    b_o = np.asarray(b_o, np.float32)

    in_maps = []
    for c in range(8):
        b, hf = divmod(c, 2)
        sl = slice(hf * HALF, (hf + 1) * HALF)
        in_maps.append({
            "qT": q[b].T.astype(BF),
            "kT": k[b].T.astype(BF),
            "vT": v[b].T.astype(BF),
            "wq": w_q[sl, :].T.astype(BF),
            "wk": w_k[sl, :].T.astype(BF),
            "wv": w_v[sl, :].T.astype(BF),
            "wo": w_o[:, sl].T.astype(BF),
            "bqc": b_q[sl].reshape(DT, 128).T.copy(),
        })

    kwargs = {}
    if TRACE:
        kwargs = dict(trace=True, trace_cores=[0])
    try:
        res = run_bass_kernel_spmd(nc, in_maps, core_ids=list(range(8)), **kwargs)
    except Exception:
        # transient device wedge (e.g. a previously killed client left a core
        # dirty) usually clears on retry
        time.sleep(2.0)
        res = run_bass_kernel_spmd(nc, in_maps, core_ids=list(range(8)), **kwargs)
    if TRACE:
        LAST_EXEC_NS = res.exec_time_ns
        LAST_TRACE = res.instructions_and_trace[1] if res.instructions_and_trace else None

    # b_v folds to a constant through the attention (rows sum to 1) and the
    # row-parallel out-projection: add b_v @ w_o.T once on the host.
    const = b_o + b_v @ w_o.T
    out = np.empty((B, S, D), np.float32)
    for b in range(B):
        out[b] = (res.results[2 * b]["out"].astype(np.float32)
                  + res.results[2 * b + 1]["out"].astype(np.float32)
                  + const[None, :])
    return out
